# revision 9
# baseline (speedup 1.0000x reference)
"""GCNConv + PReLU + L2-normalize + global_mean_pool on 8 trn2 NeuronCores.

Strategy (per sharding hint): nodes are partitioned across the 8 cores
(load-balanced by in-degree so every 128-node tile has a bounded edge
count); edges are partitioned by destination node.  Each core computes
g = dinv * (x @ W) for its node shard (x^T loaded via DMA-transpose so
one 128-wide matmul per tile suffices), the g-table is AllGathered
(bf16) to every core's HBM, then each core processes destination tiles
in blocks: row-gathers (InstDMAGatherAnt) batched over (tile-block,
table-segment) groups and rotated across 4 SWDGE queues so the random
256B row fetches run in parallel DMA streams (~2ns/row vs ~10ns on one
queue), a one-hot scatter matmul chain per tile into a per-block PSUM strip,
and a block-batched epilogue.  Per-graph partial sums are AllReduced
and divided by the per-graph node counts.

Math identities used:
- with deg = in_degree + 1 and dinv = deg^-1/2,
    out[d] = dinv[d] * ( sum_{s->d} dinv[s]*h[s] + dinv[d]*h[d] ) + b
  so the per-edge norm dinv[s]*dinv[d] is never materialized.
- when b == 0 and prelu_a is a uniform positive slope (true for these
  inputs; checked on host at runtime with a generic fallback path):
  prelu is positive-homogeneous and F.normalize is scale-invariant, so
  the outer dinv[d] scale and the bias add drop out of the epilogue and
  PReLU collapses to one Prelu activation instruction.

Perf notes (measured on 8-core trn2, 2026-08-11 session):
- The gather drain is DESCRIPTOR-RATE bound: ~160ns per 16-index
  descriptor per SWDGE queue (~10ns/idx/queue), linear in 1/queues
  (q1=2009us, q2=1125us, q4=616us full-kernel marginal), capped at 4
  queues by ucode.  It is NOT byte-bound (512B elems drain at the same
  desc rate as 256B) and NOT access-order-bound (shuffling indices
  changes nothing).  Loop-body floor with per-edge gathers is
  idx_count x 2.5ns; compute chain (spmm-only) is ~348us and hides
  under the drain.
- collectives (AllGather/AllReduce) must stay OUTSIDE any For_i repeat
  loop and coexist fine with num_swdge_queues=4.
- phase 1: dma_start_transpose moves 8 tiles per instr, 8 matmuls into
  a 2-bank PSUM strip, one broadcast dinv multiply; the gshard write
  uses the Act-engine HWDGE (nc.scalar.dma_start) to stay off SWDGE
  queue 0 which the gather drains contend for (590 -> 562us A/B).
- epilogue is batched per 3-tile block: one [P, BLKT*D] PSUM strip
  (accumulation regions per tile via out slices + skip_group_check),
  one DVE add of the self term, one Prelu over the block, squares +
  tensor_reduce(X) for the per-node norms, and one DVE broadcast
  multiply for the final scale (616 -> ~590us; frees ~140us of Act
  engine time vs the per-tile epilogue).
- chunk count sets descriptor count: the capacity repack (512/seg
  cap) plus a bounded swap-refinement pass lands ~1780 chunks vs the
  1563 ideal; per-seg totals are ~99.7% of 98x512 capacity so most
  remaining q=5 cells are a feasibility limit, not a heuristic gap.
- known dead ends (all HW-verified faults/hangs): num_idxs_reg <
  num_idxs works for ONE call per queue but the NEXT gather on that
  queue faults the device (ucode ring desync), so padding cannot be
  skipped via negative indices; num_idxs_reg from a register
  (value_load) faults immediately; indirect_dma_start hangs the
  worker; single_packet=True hangs under multi-queue + collectives;
  mid-call negative indices DO work data-wise (slots preserved,
  descriptors skipped) but only until the next call on the queue.
- idx16 rows 16..127 are 8 replicas of rows 0..15 (per-DMA-engine
  banks); any host-side index edit must be re-tiled to all replicas.
"""

import numpy as np
import ml_dtypes

import concourse.bass as bass
import concourse.bacc as bacc
import concourse.tile as tile
import concourse.mybir as mybir
from concourse.tile_rust import add_dep_helper
from concourse.bass_utils import run_bass_kernel_spmd

P = 128          # partitions / tile size
D = 128          # feature dim
G = 128          # number of graphs
NCORES = 8
MAXSEG = 25088   # gather-table segment rows (must stay < 32768 for int16)

F32 = mybir.dt.float32
BF16 = mybir.dt.bfloat16
I16 = mybir.dt.int16
AF = mybir.ActivationFunctionType
OP = mybir.AluOpType

# tuned knobs
BLKT = 3         # destination tiles per gather/one-hot block
QCAP = 16        # max 128-slot chunks per dma_gather call (2048 indices)
SCRATCH = 32768  # SWDGE descriptor carveout (2048-desc ring)
NQUEUES = 4      # SWDGE queues; gather calls rotate across them


# ----------------------------------------------------------------------------
# Host-side packing: node permutation, edge partitioning, per-core arrays.
# ----------------------------------------------------------------------------
def pack_inputs(x, edge_index, batch, BLKT=BLKT):
    N = x.shape[0]
    E = edge_index.shape[1]
    src = edge_index[0].astype(np.int64)
    dst = edge_index[1].astype(np.int64)

    ntiles = -(-(-(-N // P)) // NCORES) * NCORES
    TPC = ntiles // NCORES
    NPC = TPC * P
    NPAD = NCORES * NPC

    indeg = np.bincount(dst, minlength=N).astype(np.int64)
    deg = (indeg + 1).astype(np.float32)

    # ---- node -> (tile, slot): snake over tiles in descending in-degree ----
    order = np.argsort(-indeg, kind="stable")
    nrounds = NPAD // ntiles
    tile_seq = np.arange(ntiles)
    snake = np.empty((nrounds, ntiles), np.int64)
    snake[0::2] = tile_seq
    snake[1::2] = tile_seq[::-1]
    tile_of_slot = snake.reshape(-1)
    p_of_slot = np.repeat(np.arange(nrounds), ntiles)
    node_of_slot = np.full(NPAD, -1, np.int64)
    node_of_slot[:N] = order

    load = np.zeros(ntiles, np.int64)
    np.add.at(load, tile_of_slot[:N], indeg[order])

    # ---- tile -> (core, tidx): snake over cores in descending load ----
    tord = np.argsort(-load, kind="stable")
    core_of_tile = np.empty(ntiles, np.int64)
    tidx_of_tile = np.empty(ntiles, np.int64)
    cseq = np.arange(NCORES)
    for r in range(TPC):
        cs = cseq if r % 2 == 0 else cseq[::-1]
        tr = tord[r * NCORES:(r + 1) * NCORES]
        core_of_tile[tr] = cs
        tidx_of_tile[tr] = r

    row_of_slot = (core_of_tile[tile_of_slot] * NPC
                   + tidx_of_tile[tile_of_slot] * P + p_of_slot)
    row_of_node = np.empty(N, np.int64)
    real = node_of_slot >= 0
    row_of_node[node_of_slot[real]] = row_of_slot[real]

    NSEG = max(1, -(-NPAD // MAXSEG))
    SEG = -(-NPAD // NSEG)
    assert SEG <= 32767

    # ---- capacity-targeted repack within each core ----------------------
    # q[t,s] = ceil(max_c cnt[c,t,s]/128) taxes every core with the worst
    # core's chunk count, and the snake only balances TOTAL degree.  An
    # edge's segment is its source's core PAIR (seg = row//25088, NPC*2 =
    # 25088), which is invariant under moving nodes BETWEEN TILES OF THE
    # SAME CORE.  So: keep node->core fixed, recompute each node's
    # per-segment in-degree, greedily re-pack each core's nodes into
    # tiles capped at 512 edges/segment (q=4), and sort tiles by chunk
    # profile so expensive tiles share a tidx across cores.
    core_of_node = row_of_node // NPC
    pair_of_src = core_of_node[src] // 2
    dvec = np.zeros((N, NSEG), np.int64)
    np.add.at(dvec, (dst, pair_of_src), 1)

    row_of_node = np.empty(N, np.int64)
    for c in range(NCORES):
        nodes_c = np.where(core_of_node == c)[0]
        dv = dvec[nodes_c]
        order_c = np.argsort(-dv.sum(1), kind="stable")
        bins = np.zeros((TPC, NSEG), np.int64)
        nitems = np.zeros(TPC, np.int64)
        assign = np.empty(len(nodes_c), np.int64)
        for oi in order_c:
            d = dv[oi]
            cand = np.where(nitems < P)[0]
            over = np.maximum(bins[cand] + d - 512, 0).sum(1)
            fits = over == 0
            if fits.any():
                slack = (512 - (bins[cand] + d)).min(1)
                slack[~fits] = 1 << 30
                b = cand[np.argmin(slack)]
            else:
                b = cand[np.argmin(over)]
            assign[oi] = b
            bins[b] += d
            nitems[b] += 1
        # ---- swap-refinement: every bin holds exactly 128 nodes, so
        # only swaps can move load.  Greedy endgame spills ~60% of
        # (bin, seg) cells past the 512 cap (q 4->5, +15% gather
        # descriptors); sweeps of 1-for-1 swaps that reduce total
        # overflow reclaim most of it.
        members = [list(np.where(assign == b)[0]) for b in range(TPC)]

        def qcost(v):
            return int((-(-v // P)).sum())

        import time as _time
        _t_ref = _time.time()
        for _sweep in range(6):
            over_cells = np.argwhere(bins > 512)
            if len(over_cells) == 0 or _time.time() - _t_ref > 5:
                break
            improved = False
            for b, s in over_cells:
                if bins[b, s] <= 512:
                    continue
                mb = members[b]
                dx = dv[mb]                       # [nb, NSEG]
                xo = np.argsort(-dx[:, s])
                done = False
                for xi in xo[:12]:
                    if dx[xi, s] == 0:
                        break
                    xn = mb[xi]
                    # targets with the most room in seg s first
                    order_b2 = np.argsort(bins[:, s])
                    for b2 in order_b2[:16]:
                        if b2 == b:
                            continue
                        mb2 = members[b2]
                        dy = dv[mb2]
                        # partner with the least load in seg s
                        for yo in np.argsort(dy[:, s])[:2]:
                            yo = int(yo)
                            yn = mb2[yo]
                            nb = bins[b] - dv[xn] + dv[yn]
                            nb2 = bins[b2] - dv[yn] + dv[xn]
                            oldc = qcost(bins[b]) + qcost(bins[b2])
                            newc = qcost(nb) + qcost(nb2)
                            if newc < oldc:
                                bins[b], bins[b2] = nb, nb2
                                members[b][xi] = yn
                                members[b2][yo] = xn
                                assign[xn], assign[yn] = b2, b
                                done = True
                                improved = True
                                break
                        if done:
                            break
                    if done:
                        break
            if not improved:
                break
        # align chunk profiles across cores: heavy tiles first
        qv = -(-bins // P)
        keys = np.concatenate([qv, bins], axis=1)
        tile_rank = sorted(range(TPC),
                           key=lambda i: tuple(keys[i]), reverse=True)
        new_tidx = np.empty(TPC, np.int64)
        new_tidx[tile_rank] = np.arange(TPC)
        # slot order within a tile is arbitrary
        slot_ctr = np.zeros(TPC, np.int64)
        for oi in order_c:
            b = assign[oi]
            row_of_node[nodes_c[oi]] = (c * NPC + new_tidx[b] * P
                                        + slot_ctr[b])
            slot_ctr[b] += 1

    node_at_row = np.full(NCORES * NPC, -1, np.int64)
    node_at_row[row_of_node[np.arange(N)]] = np.arange(N)

    # ---- per-(core, dst tile, src segment) edge chunk counts ----
    er = row_of_node[dst]
    ecore = er // NPC
    etile = (er % NPC) // P
    ep = er % P
    esrc = row_of_node[src]
    eseg = esrc // SEG

    key = (ecore * TPC + etile) * NSEG + eseg
    cnts = np.bincount(key, minlength=NCORES * TPC * NSEG)
    cnts = cnts.reshape(NCORES, TPC, NSEG)
    q = (-(-cnts.max(axis=0) // P)).astype(np.int64)          # [TPC, NSEG]

    # ---- chunk layout: for block i: for seg s: for t in block ----
    NBLK = -(-TPC // BLKT)
    chunk_off = np.zeros((TPC, NSEG), np.int64)
    call_plan = []                     # (block, seg, chunk_base, nchunks)
    blk_base = np.zeros(NBLK + 1, np.int64)
    pos = 0
    for i in range(NBLK):
        t0, t1 = i * BLKT, min((i + 1) * BLKT, TPC)
        blk_base[i] = pos
        for s in range(NSEG):
            base = pos
            for t in range(t0, t1):
                chunk_off[t, s] = pos
                pos += q[t, s]
            if pos > base:
                call_plan.append((i, s, base, pos - base))
        blk_base[i + 1] = pos
    CHK = int(pos)
    BCH = int((blk_base[1:] - blk_base[:-1]).max())

    # ---- place edges into chunk slots ----
    eo = np.lexsort((esrc, key))
    key_s = key[eo]
    srow_s = esrc[eo]
    ep_s = ep[eo]
    gs = np.searchsorted(key_s, np.arange(NCORES * TPC * NSEG))
    i_in = np.arange(E) - gs[key_s]
    c2 = key_s // (TPC * NSEG)
    t2 = (key_s // NSEG) % TPC
    s2 = key_s % NSEG
    cb = chunk_off[t2, s2]

    dstloc = np.full((NCORES, P, CHK), 255.0, ml_dtypes.bfloat16)
    dstloc[c2, i_in % P, cb + i_in // P] = ep_s.astype(ml_dtypes.bfloat16)
    # Padding slots must fetch SOME valid row (their dst one-hot is 255 so
    # they contribute nothing).  Spread them across table rows: all-same-row
    # padding funnels into one HBM bank and serializes the DMA drain.
    pad = ((np.arange(CHK * 8, dtype=np.int64)[None, :] * 97
            + np.arange(16, dtype=np.int64)[:, None] * 1567)
           % SEG).astype(np.int16)
    idx16 = np.broadcast_to(pad, (NCORES, 16, CHK * 8)).copy()
    idx16[c2, i_in % 16, cb * 8 + i_in // 16] = (
        srow_s - s2 * SEG).astype(np.int16)
    idx16 = np.tile(idx16, (1, 8, 1))

    # ---- per-core node arrays ----
    nar = node_at_row.reshape(NCORES, NPC)
    xsh = np.zeros((NCORES, NPC, D), ml_dtypes.bfloat16)
    degsh = np.ones((NCORES, P, TPC), np.float32)
    batsh = np.zeros((NCORES, P, TPC), ml_dtypes.bfloat16)
    for c in range(NCORES):
        nc_ = nar[c]
        m = nc_ >= 0
        xsh[c][m] = x[nc_[m]].astype(ml_dtypes.bfloat16)
        dg = np.ones(NPC, np.float32)
        dg[m] = deg[nc_[m]]
        degsh[c] = dg.reshape(TPC, P).T
        bt = np.full(NPC, 255.0, np.float32)
        bt[m] = batch[nc_[m]].astype(np.float32)
        batsh[c] = bt.reshape(TPC, P).T.astype(ml_dtypes.bfloat16)

    cnt = np.bincount(batch.astype(np.int64), minlength=G).astype(np.float32)

    return dict(TPC=TPC, NPC=NPC, NPAD=NPAD, NSEG=NSEG, SEG=SEG,
                BLKT=BLKT, NBLK=NBLK, CHK=CHK, BCH=BCH,
                q=q, chunk_off=chunk_off, blk_base=blk_base,
                call_plan=call_plan,
                idx16=idx16, dstloc=dstloc, xsh=xsh, degsh=degsh,
                batsh=batsh, cnt=cnt.reshape(G, 1))


def host_flags(b, prelu_a):
    b = np.asarray(b, np.float32)
    a = np.asarray(prelu_a, np.float32)
    fast = bool(np.all(b == 0.0) and np.all(a == a.reshape(-1)[0])
                and a.reshape(-1)[0] > 0)
    return fast, float(a.reshape(-1)[0])


# ----------------------------------------------------------------------------
# Device program.
# ----------------------------------------------------------------------------
def build_program(pk, repeats=1, mode="full", QCAP=QCAP, scratch=SCRATCH,
                  single_packet=False, nqueues=NQUEUES, alpha=0.25,
                  fastpath=True):
    TPC, NPAD, NSEG, SEG = pk["TPC"], pk["NPAD"], pk["NSEG"], pk["SEG"]
    NPC = TPC * P
    CHK, BCH, NBLK, BLKT = pk["CHK"], pk["BCH"], pk["NBLK"], pk["BLKT"]
    q, chunk_off, blk_base = pk["q"], pk["chunk_off"], pk["blk_base"]
    call_plan = pk["call_plan"]

    nc = bacc.Bacc("TRN2", target_bir_lowering=False, debug=False,
                   num_devices=NCORES, dynamic_dma_scratch_size=scratch,
                   num_swdge_queues=nqueues)

    xsh = nc.dram_tensor("xsh", [NPC, D], BF16, kind="ExternalInput")
    degsh = nc.dram_tensor("degsh", [P, TPC], F32, kind="ExternalInput")
    batsh = nc.dram_tensor("batsh", [P, TPC], BF16, kind="ExternalInput")
    idx16 = nc.dram_tensor("idx16", [P, CHK * 8], I16, kind="ExternalInput")
    dstloc = nc.dram_tensor("dstloc", [P, CHK], BF16, kind="ExternalInput")
    w_in = nc.dram_tensor("w", [D, D], F32, kind="ExternalInput")
    b_in = nc.dram_tensor("b", [1, D], F32, kind="ExternalInput")
    a_in = nc.dram_tensor("a", [1, D], F32, kind="ExternalInput")
    cnt_in = nc.dram_tensor("cnt", [G, 1], F32, kind="ExternalInput")
    pooled_out = nc.dram_tensor("pooled", [G, D], F32, kind="ExternalOutput")

    gshard = nc.dram_tensor("gshard", [NPC, D], BF16)
    gtable = nc.dram_tensor("gtable", [NPAD, D], BF16, addr_space="Shared")
    ar_in = nc.dram_tensor("ar_in", [G, D], F32)
    ar_out = nc.dram_tensor("ar_out", [G, D], F32, addr_space="Shared")

    with tile.TileContext(nc, num_cores=NCORES) as tc:
        with (
            tc.tile_pool(name="const", bufs=1) as constp,
            tc.tile_pool(name="resident", bufs=1) as resp,
            tc.tile_pool(name="meta", bufs=1) as metap,
        ):
            w_t32 = constp.tile([D, D], F32)
            nc.sync.dma_start(out=w_t32[:], in_=w_in[:])
            w_t = constp.tile([D, D], BF16)
            nc.vector.tensor_copy(w_t[:], w_t32[:])
            brow = constp.tile([1, D], F32)
            nc.sync.dma_start(out=brow[:], in_=b_in[:])
            bbc = constp.tile([P, D], F32)
            nc.gpsimd.partition_broadcast(bbc[:], brow[:])
            arow = constp.tile([1, D], F32)
            nc.sync.dma_start(out=arow[:], in_=a_in[:])
            abc = constp.tile([P, D], F32)
            nc.gpsimd.partition_broadcast(abc[:], arow[:])
            iota_dst = constp.tile([P, BCH * P], BF16)
            nc.gpsimd.iota(iota_dst[:], pattern=[[0, BCH], [1, P]], base=0,
                           channel_multiplier=0,
                           allow_small_or_imprecise_dtypes=True)
            cntc = constp.tile([G, 1], F32)
            nc.sync.dma_start(out=cntc[:], in_=cnt_in[:])
            rcnt = constp.tile([G, 1], F32)
            nc.vector.tensor_scalar_max(rcnt[:], cntc[:], 1.0)
            nc.vector.reciprocal(rcnt[:], rcnt[:])

            idx_t = metap.tile([P, CHK * 8], I16)
            nc.sync.dma_start(out=idx_t[:], in_=idx16[:])
            dst_t = metap.tile([P, CHK], BF16)
            nc.sync.dma_start(out=dst_t[:], in_=dstloc[:])
            bat_t = metap.tile([P, TPC], BF16)
            nc.sync.dma_start(out=bat_t[:], in_=batsh[:])
            deg_t = metap.tile([P, TPC], F32)
            nc.sync.dma_start(out=deg_t[:], in_=degsh[:])
            dinv = resp.tile([P, TPC], F32)
            nc.scalar.sqrt(dinv[:], deg_t[:])
            nc.vector.reciprocal(dinv[:], dinv[:])
            gres = resp.tile([P, TPC * D], BF16)
            pooled_sb = resp.tile([G, D], F32)
            red_sb = resp.tile([G, D], F32)
            fin_sb = resp.tile([G, D], F32)
            # pool one-hots for all tiles, one instr (iota freed after)
            pooloh = resp.tile([P, TPC * P], BF16)
            with tc.tile_pool(name="setup_tmp", bufs=1) as stp:
                iota_gr = stp.tile([P, TPC * P], BF16)
                nc.gpsimd.iota(iota_gr[:], pattern=[[0, TPC], [1, P]],
                               base=0, channel_multiplier=0,
                               allow_small_or_imprecise_dtypes=True)
                nc.vector.tensor_tensor(
                    out=pooloh[:],
                    in0=bat_t[:, :].to_broadcast([P, TPC, P]),
                    in1=iota_gr[:], op=OP.is_equal)

            # ============ phase 1: g = dinv * (x @ W), bf16 ============
            # 8 tiles per step: one [1024,128]->[128,1024] transpose DMA,
            # 8 matmuls into a 2-bank PSUM strip, one broadcast-multiply
            # by dinv, one 8-tile gshard write.
            P1B = 8

            def phase1():
                with (
                    tc.tile_pool(name="p1x", bufs=3) as p1x,
                    tc.tile_pool(name="p1hps", bufs=2, space="PSUM") as p1hps,
                ):
                    for t0 in range(0, TPC, P1B):
                        nn = min(P1B, TPC - t0)
                        xT = p1x.tile([D, P1B * P], BF16, tag="xT")
                        nc.sync.dma_start_transpose(
                            xT[:, :nn * P],
                            xsh[t0 * P:(t0 + nn) * P, :])
                        h_ps = p1hps.tile([P, P1B * D], F32)
                        for k in range(nn):
                            nc.tensor.matmul(
                                out=h_ps[:, k * D:(k + 1) * D],
                                lhsT=xT[:, k * P:(k + 1) * P],
                                rhs=w_t[:], start=True, stop=True)
                        nc.vector.tensor_tensor(
                            out=gres[:, t0 * D:(t0 + nn) * D]
                            .rearrange("p (j d) -> p j d", j=nn),
                            in0=h_ps[:, :nn * D]
                            .rearrange("p (j d) -> p j d", j=nn),
                            in1=dinv[:, t0:t0 + nn]
                            .to_broadcast([P, nn, D]),
                            op=OP.mult)
                        # Act-engine HWDGE keeps this write off SWDGE
                        # queue 0, which the gather drains contend for
                        nc.scalar.dma_start(
                            out=gshard[t0 * P:(t0 + nn) * P, :]
                            .rearrange("(j p) d -> p j d", p=P),
                            in_=gres[:, t0 * D:(t0 + nn) * D]
                            .rearrange("p (j d) -> p j d", j=nn))

            def do_ag():
                return nc.gpsimd.collective_compute(
                    "AllGather", OP.bypass,
                    replica_groups=[list(range(NCORES))],
                    ins=[gshard[:]], outs=[gtable[:]],
                )

            def emit_gathers(i, gt, cc_ag):
                b0 = int(blk_base[i])
                ci = 0
                for (bi, s, base, n) in call_plan:
                    if bi != i:
                        continue
                    a = 0
                    while a < n:
                        bb = min(a + QCAP, n)
                        o = base - b0 + a
                        gi = nc.gpsimd.dma_gather(
                            gt[:, o * D:(o + bb - a) * D]
                            .rearrange("p (k d) -> p k d", k=bb - a),
                            gtable[s * SEG:min((s + 1) * SEG, NPAD), :],
                            idx_t[:, (base + a) * 8:(base + bb) * 8],
                            (bb - a) * P, (bb - a) * P, D,
                            elem_step=D, single_packet=single_packet,
                            queue_num=ci % nqueues)
                        ci += 1
                        if cc_ag is not None:
                            add_dep_helper(gi.ins, cc_ag.ins,
                                           reason="gather reads gtable")
                        a = bb

            # ===== phase 3: gather + one-hot scatter matmul + epilogue =====
            def phase3(cc_ag, pooled_ps):
                with (
                    tc.tile_pool(name="gat", bufs=3) as gatp,
                    tc.tile_pool(name="oh", bufs=2) as ohp,
                    tc.tile_pool(name="sps", bufs=3, space="PSUM") as spsp,
                    tc.tile_pool(name="epi", bufs=2) as epip,
                    tc.tile_pool(name="sm", bufs=3) as smp,
                ):
                    for i in range(NBLK):
                        t0, t1 = i * BLKT, min((i + 1) * BLKT, TPC)
                        b0 = int(blk_base[i])
                        bn = int(blk_base[i + 1]) - b0
                        gt = gatp.tile([P, BCH * D], BF16, tag="gt")
                        emit_gathers(i, gt, cc_ag)
                        oh = ohp.tile([P, BCH * P], BF16, tag="oh")
                        nc.vector.tensor_tensor(
                            out=oh[:, :bn * P],
                            in0=dst_t[:, b0:b0 + bn]
                                .to_broadcast([P, bn, P]),
                            in1=iota_dst[:, :bn * P], op=OP.is_equal)
                        nt = t1 - t0
                        nd = nt * D
                        s_ps = spsp.tile([P, BLKT * D], F32, tag="s")
                        for t in range(t0, t1):
                            tr = t - t0
                            KE = int(q[t].sum())
                            k2 = 0
                            for s in range(NSEG):
                                qq = int(q[t, s])
                                co = int(chunk_off[t, s]) - b0
                                for k in range(qq):
                                    nc.tensor.matmul(
                                        out=s_ps[:, tr * D:(tr + 1) * D],
                                        lhsT=oh[:, (co + k) * P:
                                                (co + k + 1) * P],
                                        rhs=gt[:, (co + k) * D:
                                               (co + k + 1) * D],
                                        start=(k2 == 0),
                                        stop=(k2 == KE - 1),
                                        skip_group_check=True)
                                    k2 += 1
                        # ---- batched epilogue over the block's nt tiles ----
                        u = epip.tile([P, BLKT * D], F32, tag="u")
                        nc.vector.tensor_tensor(
                            out=u[:, :nd], in0=s_ps[:, :nd],
                            in1=gres[:, t0 * D:t1 * D], op=OP.add)
                        v = epip.tile([P, BLKT * D], F32, tag="v")
                        if fastpath:
                            # b==0, uniform slope: dinv scale and bias drop
                            # out (prelu is +-homogeneous and the L2
                            # normalize is scale-invariant)
                            nc.scalar.activation(v[:, :nd], u[:, :nd],
                                                 AF.Prelu, alpha=alpha)
                        else:
                            nc.vector.tensor_tensor(
                                out=u[:, :nd].rearrange(
                                    "p (t d) -> p t d", t=nt),
                                in0=u[:, :nd].rearrange(
                                    "p (t d) -> p t d", t=nt),
                                in1=dinv[:, t0:t1].to_broadcast([P, nt, D]),
                                op=OP.mult)
                            nc.vector.tensor_tensor(
                                out=u[:, :nd].rearrange(
                                    "p (t d) -> p t d", t=nt),
                                in0=u[:, :nd].rearrange(
                                    "p (t d) -> p t d", t=nt),
                                in1=bbc[:].to_broadcast([P, nt, D]),
                                op=OP.add)
                            pos = epip.tile([P, BLKT * D], F32, tag="pos")
                            nc.scalar.activation(pos[:, :nd], u[:, :nd],
                                                 AF.Relu)
                            neg = epip.tile([P, BLKT * D], F32, tag="neg")
                            nc.vector.tensor_tensor(
                                out=neg[:, :nd], in0=u[:, :nd],
                                in1=pos[:, :nd], op=OP.subtract)
                            nc.vector.tensor_tensor(
                                out=neg[:, :nd].rearrange(
                                    "p (t d) -> p t d", t=nt),
                                in0=neg[:, :nd].rearrange(
                                    "p (t d) -> p t d", t=nt),
                                in1=abc[:].to_broadcast([P, nt, D]),
                                op=OP.mult)
                            nc.vector.tensor_tensor(
                                out=v[:, :nd], in0=pos[:, :nd],
                                in1=neg[:, :nd], op=OP.add)
                        nc.vector.tensor_tensor(
                            out=u[:, :nd], in0=v[:, :nd],
                            in1=v[:, :nd], op=OP.mult)
                        ss = smp.tile([P, BLKT], F32, tag="ss")
                        nc.vector.tensor_reduce(
                            out=ss[:, :nt],
                            in_=u[:, :nd].rearrange(
                                "p (t d) -> p t d", t=nt),
                            axis=mybir.AxisListType.X, op=OP.add)
                        nc.scalar.sqrt(ss[:, :nt], ss[:, :nt])
                        nc.vector.tensor_scalar_max(ss[:, :nt],
                                                    ss[:, :nt], 1e-12)
                        nc.vector.reciprocal(ss[:, :nt], ss[:, :nt])
                        o3 = epip.tile([P, BLKT * D], BF16, tag="o3")
                        nc.vector.tensor_tensor(
                            out=o3[:, :nd].rearrange(
                                "p (t d) -> p t d", t=nt),
                            in0=v[:, :nd].rearrange(
                                "p (t d) -> p t d", t=nt),
                            in1=ss[:, :nt].to_broadcast([P, nt, D]),
                            op=OP.mult)
                        for t in range(t0, t1):
                            tr = t - t0
                            nc.tensor.matmul(
                                out=pooled_ps[:],
                                lhsT=pooloh[:, t * P:(t + 1) * P],
                                rhs=o3[:, tr * D:(tr + 1) * D],
                                start=(t == 0),
                                stop=(t == TPC - 1),
                                skip_group_check=True)
                nc.vector.tensor_copy(pooled_sb[:], pooled_ps[:])

            def fin_block():
                nc.sync.dma_start(out=ar_in[:], in_=pooled_sb[:])
                nc.gpsimd.collective_compute(
                    "AllReduce", OP.add,
                    replica_groups=[list(range(NCORES))],
                    ins=[ar_in[:]], outs=[ar_out[:]],
                )
                nc.sync.dma_start(out=red_sb[:], in_=ar_out[:])
                nc.scalar.mul(fin_sb[:], red_sb[:], rcnt[:])
                nc.sync.dma_start(out=pooled_out[:], in_=fin_sb[:])

            with tc.tile_pool(name="poolacc", bufs=1,
                              space="PSUM") as poolaccp:
                pooled_ps = poolaccp.tile([G, D], F32)
                phase1()
                cc = do_ag()
                phase3(cc, pooled_ps)
                if repeats > 1:
                    # timing loop: collectives stay outside (a collective
                    # inside For_i desyncs the mesh)
                    with tc.For_i(0, repeats - 1, 1):
                        phase1()
                        phase3(cc, pooled_ps)
            fin_block()

    nc.compile()
    return nc


def make_in_maps(pk, W, b, prelu_a):
    W = np.ascontiguousarray(W, np.float32)
    b = np.ascontiguousarray(b, np.float32).reshape(1, D)
    a = np.ascontiguousarray(prelu_a, np.float32).reshape(1, D)
    return [
        {
            "xsh": pk["xsh"][c], "degsh": pk["degsh"][c],
            "batsh": pk["batsh"][c], "idx16": pk["idx16"][c],
            "dstloc": pk["dstloc"][c],
            "w": W, "b": b, "a": a, "cnt": pk["cnt"],
        }
        for c in range(NCORES)
    ]


def kernel(x, edge_index, batch, W, b, prelu_a):
    x = np.asarray(x)
    edge_index = np.asarray(edge_index)
    batch = np.asarray(batch)
    pk = pack_inputs(x, edge_index, batch)
    fast, alpha = host_flags(b, prelu_a)
    nc = build_program(pk, repeats=1, alpha=alpha, fastpath=fast)
    in_maps = make_in_maps(pk, np.asarray(W), np.asarray(b),
                           np.asarray(prelu_a))
    res = run_bass_kernel_spmd(nc, in_maps, core_ids=list(range(NCORES)))
    return np.asarray(res.results[0]["pooled"], np.float32)

